# revision 1
# baseline (speedup 1.0000x reference)
"""Trainium2 Bass kernel for nn_ConsolidationModel.

Mathematical reduction (verified bit-exact against the reference scan):
the scan's control flow is data-independent (count depends only on t).
Consolidation fires at t=15/31/47, but between consecutive firings the
8-slot FIFO receives 4 appends + 12 shift-appends, which evicts every
consolidated row before the next firing — and after the last firing
(t=47) there are 4 appends + 11 shifts, so at t=62 the buffer holds
exactly the embeddings of tokens 55..62 with count=8.  The model output
is therefore:

    mem  = mean_p embed[seqs[:, 55+p]]          (p = 0..7)
    h    = concat([embed[query_tok], mem], -1)  (B, 128)
    out  = relu(h @ r1_w.T + r1_b) @ r2_w.T + r2_b

Device algorithm (per core, batch shard of 256 rows; vocab/hidden on
partitions, batch on the free axis — 11 instructions total):

  wide (64, 2304) i16 <- ONE DMA with a stride-0 partition AP: the 9
                         token rows (8 tail positions + query)
                         broadcast across the 64 vocab partitions
  m8    = (wide[:, :2048] == iota)  bf16 one-hot masks        (1 DVE op)
  histT = bf16 add-tree over the 8 position masks             (3 DVE ops)
  qT    = (wide[:, 2048:] == iota)  f32                       (1 DVE op)
  hidT  = [B; A]^T @ [histT; qT]   ONE K=128 matmul, where
          A = embed @ r1_w[:, :64].T, B = embed @ r1_w[:, 64:].T / 8
          are folded on the host (data-independent weight prep)
  hid   = relu(hidT + r1_b)        one 2-op tensor_scalar
  logT  = r2_w.T^T @ hid           K=64 matmul
  out   = logT + r2_b              tensor_scalar add (PSUM -> SBUF)
  -> DMA out (64, 256); the host transposes each shard back to
     (256, 64) while gathering the 8 shards.

Sharding: pure data parallel over batch across 8 cores; parameters
replicated.
"""

import numpy as np

N_CORES = 8
B = 2048           # full batch
BS = B // N_CORES  # 256 per-core batch shard
H = 64             # hidden dim
V = 64             # vocab
TAIL_LO, TAIL_HI = 55, 63  # token positions that survive in the buffer
NPOS = TAIL_HI - TAIL_LO   # 8

_compiled_nc = None


def _build_program():
    import concourse.bacc as bacc
    import concourse.mybir as mybir
    from concourse import tile

    f32 = mybir.dt.float32
    bf16 = mybir.dt.bfloat16
    u8 = mybir.dt.uint8
    eq = mybir.AluOpType.is_equal
    add = mybir.AluOpType.add
    mx = mybir.AluOpType.max

    nc = bacc.Bacc("TRN2", target_bir_lowering=False, debug=False,
                   num_devices=N_CORES)

    i16 = mybir.dt.int16
    toks_d = nc.declare_dram_parameter("toks", [1, 9 * BS], i16, isOutput=False)
    cst_d = nc.declare_dram_parameter("cst", [2 * H, 131], f32, isOutput=False)
    out_d = nc.declare_dram_parameter("logT", [V, BS], f32, isOutput=True)

    with tile.TileContext(nc) as tc:
        with (
            tc.tile_pool(name="sb", bufs=1) as pool,
            tc.tile_pool(name="ps", bufs=1, space="PSUM") as pp,
        ):
            # iota column generated on-device: no DMA dependency for the eqs
            iota_t = pool.tile([V, 1], f32)
            nc.gpsimd.iota(iota_t[:], pattern=[[0, 1]], base=0,
                           channel_multiplier=1,
                           allow_small_or_imprecise_dtypes=True)
            iota = iota_t[:, 0:1]

            # token broadcast: stride-0 partition AP, int16 for the DVE
            # 2x mode
            wide = pool.tile([V, 9 * BS], i16)
            nc.sync.dma_start(wide[:], toks_d[:].to_broadcast((V, 9 * BS)))
            cst = pool.tile([2 * H, 131], f32)
            nc.scalar.dma_start(cst[:], cst_d[:])
            r1b = cst[0:H, 128:129]
            r2b = cst[0:V, 129:130]

            # one-hot masks + histogram (bf16 add tree; counts <= 8 exact)
            hq = pool.tile([2 * H, BS], f32)   # rows 0:64 histT, 64:128 qT
            m8 = pool.tile([V, NPOS * BS], bf16)
            nc.vector.tensor_scalar(m8[:], wide[:, 0:NPOS * BS], iota, None, eq)
            s2 = pool.tile([V, 4 * BS], bf16)
            nc.vector.tensor_add(s2[:], m8[:, 0:4 * BS], m8[:, 4 * BS:8 * BS])
            s4 = pool.tile([V, 2 * BS], bf16)
            nc.vector.tensor_add(s4[:], s2[:, 0:2 * BS], s2[:, 2 * BS:4 * BS])
            nc.vector.tensor_add(hq[0:V, :], s4[:, 0:BS], s4[:, BS:2 * BS])
            nc.vector.tensor_scalar(hq[V:2 * V, :], wide[:, NPOS * BS:9 * BS], iota, None, eq)

            # hidT = B^T @ histT + A^T @ qT   (single K=128 matmul)
            hidT_ps = pp.tile([H, BS], f32, tag="hid")
            nc.tensor.matmul(hidT_ps[:], cst[:, 0:64], hq[:], start=True, stop=True)
            # hid = relu(hidT + r1_b)   (scalar engine, off the DVE)
            hid = pool.tile([H, BS], f32)
            nc.scalar.activation(hid[:], hidT_ps[:],
                                 mybir.ActivationFunctionType.Relu,
                                 bias=r1b, scale=1.0)

            # logT = r2wT^T @ hid ; + r2_b on the PSUM->SBUF move
            logT_ps = pp.tile([V, BS], f32, tag="log")
            nc.tensor.matmul(logT_ps[:], cst[0:H, 64:128], hid[:], start=True, stop=True)
            logT_sb = pool.tile([V, BS], f32)
            nc.vector.tensor_scalar(logT_sb[:], logT_ps[:], r2b, None, add)
            nc.sync.dma_start(out_d[:], logT_sb[:])

    nc.compile()
    return nc


def _prep_in_maps(inputs):
    embed = np.asarray(inputs["embed"], dtype=np.float32)[:V]      # (64, 64)
    r1_w = np.asarray(inputs["r1_w"], dtype=np.float32)            # (64, 128)
    r1_b = np.asarray(inputs["r1_b"], dtype=np.float32)            # (64,)
    r2_w = np.asarray(inputs["r2_w"], dtype=np.float32)            # (64, 64)
    r2_b = np.asarray(inputs["r2_b"], dtype=np.float32)            # (64,)
    seqs = np.asarray(inputs["seqs"])                              # (B, 64) int
    query = np.asarray(inputs["query_tok"])                        # (B,) int

    A = embed @ r1_w[:, :H].T                                      # (64v, 64h)
    Bm = (embed @ r1_w[:, H:].T) * np.float32(1.0 / NPOS)          # (64v, 64h)
    cst = np.zeros((2 * H, 131), np.float32)
    cst[0:V, 0:64] = Bm
    cst[V:2 * V, 0:64] = A
    cst[0:H, 64:128] = r2_w.T
    cst[0:H, 128] = r1_b
    cst[0:V, 129] = r2_b
    cst[0:V, 130] = np.arange(V, dtype=np.float32)

    # token rows, position-major, then regrouped into the two batch
    # halves (64 | 192) the kernel pipelines over
    toks = np.empty((N_CORES, 9, BS), np.int16)
    toks[:, :NPOS, :] = (
        seqs[:, TAIL_LO:TAIL_HI].astype(np.int16).reshape(N_CORES, BS, NPOS)
        .transpose(0, 2, 1))
    toks[:, NPOS, :] = query.astype(np.int16).reshape(N_CORES, BS)

    return [
        {"toks": toks[c].reshape(1, 9 * BS), "cst": cst}
        for c in range(N_CORES)
    ]


def kernel(**inputs):
    global _compiled_nc
    from concourse.bass_utils import run_bass_kernel_spmd

    in_maps = _prep_in_maps(inputs)
    if _compiled_nc is None:
        _compiled_nc = _build_program()
    res = run_bass_kernel_spmd(_compiled_nc, in_maps, list(range(N_CORES)))
    out = np.empty((B, V), np.float32)
    for c in range(N_CORES):
        out[c * BS:(c + 1) * BS] = res.results[c]["logT"].T
    return out


if __name__ == "__main__":
    rng = np.random.default_rng(0)
    demo = {
        "embed": rng.standard_normal((V + 2, H)).astype(np.float32),
        "r1_w": rng.standard_normal((H, 2 * H)).astype(np.float32) * 0.05,
        "r1_b": rng.standard_normal(H).astype(np.float32) * 0.02,
        "r2_w": rng.standard_normal((V, H)).astype(np.float32) * 0.05,
        "r2_b": rng.standard_normal(V).astype(np.float32) * 0.02,
        "seqs": rng.integers(0, V, (B, 64)),
        "query_tok": rng.integers(0, V, (B,)),
    }
    out = kernel(**demo)
    tail = demo["embed"][demo["seqs"][:, TAIL_LO:TAIL_HI]]
    mem = tail.sum(1) / NPOS
    h = np.concatenate([demo["embed"][demo["query_tok"]], mem], -1)
    exp = np.maximum(h @ demo["r1_w"].T + demo["r1_b"], 0) @ demo["r2_w"].T + demo["r2_b"]
    err = np.abs(out - exp).max() / np.abs(exp).max()
    print("self-check rel err:", err)



# revision 8
# speedup vs baseline: 1.0772x; 1.0772x over previous
"""Trainium2 Bass kernel for nn_ConsolidationModel.

Mathematical reduction (verified bit-exact against the reference scan):
the scan's control flow is data-independent (count depends only on t).
Consolidation fires at t=15/31/47, but between consecutive firings the
8-slot FIFO receives 4 appends + 12 shift-appends, which evicts every
consolidated row before the next firing — and after the last firing
(t=47) there are 4 appends + 11 shifts, so at t=62 the buffer holds
exactly the embeddings of tokens 55..62 with count=8.  The model output
is therefore:

    mem  = mean_p embed[seqs[:, 55+p]]          (p = 0..7)
    h    = concat([embed[query_tok], mem], -1)  (B, 128)
    out  = relu(h @ r1_w.T + r1_b) @ r2_w.T + r2_b

Device algorithm (per core, batch shard of 256 rows).  The v1 kernel
broadcast the tokens across 64 vocab partitions with a 294KB stride-0
DMA (~4.3us end-to-end) and reduced the one-hot masks with a DVE add
tree + fp32 LOW_HIGH matmuls (~6us serial).  v2 removes both:

  toks (2, 1280) bf16  <- ONE tiny 5KB DMA (sync queue): row0 = tail
                          positions 0..3 + query, row1 = positions
                          4..7 offset by +64 (so one 0..127 iota
                          serves both partition halves)
  cst  (128, 256) bf16 <- 65KB DMA (scalar queue): [Bm;Bm], A, r2w.T,
                          biases, all folded on host (data-independent)
  bcA/bcB/bcQ PSUM     <- TensorE K=2 selector matmuls broadcast the
                          token rows across 128/64 partitions (PE was
                          idle; kills the big DMA)
  m8, qT bf16          <- is_equal vs on-device iota, read from PSUM
  hidT PSUM            <- 5 bf16 matmuls accumulate the histogram
                          contraction directly (K=128 [Bm;Bm] does the
                          top+bottom position sum; no DVE add tree)
  hid  = relu(hidT+r1b)   scalar ACT, PSUM->SBUF
  logT = r2wT^T @ hid     bf16 matmul
  out  = logT + r2b       split: DVE half + ACT half, then two 32KB
                          DMAs on the sync/scalar queues in parallel

Sharding: pure data parallel over batch across 8 cores; parameters
replicated.  Host transposes each (64, 256) shard back to (256, 64).
"""

import numpy as np

N_CORES = 8
B = 2048           # full batch
BS = B // N_CORES  # 256 per-core batch shard
H = 64             # hidden dim
V = 64             # vocab
TAIL_LO, TAIL_HI = 55, 63  # token positions that survive in the buffer
NPOS = TAIL_HI - TAIL_LO   # 8

_compiled_nc = None


def _build_program():
    import concourse.bacc as bacc
    import concourse.mybir as mybir
    from concourse import tile

    f32 = mybir.dt.float32
    bf16 = mybir.dt.bfloat16
    eq = mybir.AluOpType.is_equal
    add = mybir.AluOpType.add

    nc = bacc.Bacc("TRN2", target_bir_lowering=False, debug=False,
                   num_devices=N_CORES)

    toks_d = nc.declare_dram_parameter("toks", [1, 10 * BS], bf16, isOutput=False)
    cst_d = nc.declare_dram_parameter("cst", [2 * H, 256], bf16, isOutput=False)
    out_d = nc.declare_dram_parameter("logT", [V, BS], f32, isOutput=True)

    with tile.TileContext(nc) as tc:
        with (
            tc.tile_pool(name="sb", bufs=1) as pool,
            tc.tile_pool(name="ps", bufs=1, space="PSUM") as pp,
        ):
            # on-device constants: iota column 0..127 and the K=2
            # selector matrix (top half <- row0, bottom half <- row1)
            iota_t = pool.tile([2 * V, 1], f32)
            nc.gpsimd.iota(iota_t[:], pattern=[[0, 1]], base=0,
                           channel_multiplier=1,
                           allow_small_or_imprecise_dtypes=True)
            iota = iota_t[:, 0:1]
            ones = pool.tile([1, V], bf16)
            nc.gpsimd.memset(ones[:], 1.0)

            # single-partition token row: [pos0..3 | query | pos4..7+64 | pad]
            toks = pool.tile([1, 10 * BS], bf16)
            nc.sync.dma_start(toks[:], toks_d[:])
            cst = pool.tile([2 * H, 256], bf16)
            nc.scalar.dma_start(cst[:], cst_d[:])
            # biases are f32 bit patterns packed into bf16 column pairs
            r1b = cst[0:H, 192:194].bitcast(f32)
            r2b = cst[0:V, 194:196].bitcast(f32)

            # TensorE broadcast of the token rows across partitions
            # (K=1 ones-row matmuls; PE is otherwise idle here)
            ROW1 = 5 * BS  # col offset of the +64-shifted pos4..7 row
            bcQ = pp.tile([V, BS], f32, tag="bcQ")
            nc.tensor.matmul(bcQ[:], ones[:], toks[0:1, 4 * BS:5 * BS],
                             start=True, stop=True)
            bcA = pp.tile([2 * V, 2 * BS], f32, tag="bcA")
            nc.tensor.matmul(bcA[0:V, :], ones[:], toks[0:1, 0:2 * BS],
                             start=True, stop=True)
            nc.tensor.matmul(bcA[V:2 * V, :], ones[:],
                             toks[0:1, ROW1:ROW1 + 2 * BS],
                             start=True, stop=True)
            bcB = pp.tile([2 * V, 2 * BS], f32, tag="bcB")
            nc.tensor.matmul(bcB[0:V, :], ones[:], toks[0:1, 2 * BS:4 * BS],
                             start=True, stop=True)
            nc.tensor.matmul(bcB[V:2 * V, :], ones[:],
                             toks[0:1, ROW1 + 2 * BS:ROW1 + 4 * BS],
                             start=True, stop=True)

            # one-hot masks vs iota (DVE reads PSUM directly)
            qT = pool.tile([V, BS], bf16)
            nc.vector.tensor_scalar(qT[:], bcQ[:], iota_t[0:V, 0:1], None, eq)
            m8 = pool.tile([2 * V, 4 * BS], bf16)
            nc.vector.tensor_scalar(m8[:, 0:2 * BS], bcA[:], iota, None, eq)
            nc.vector.tensor_scalar(m8[:, 2 * BS:4 * BS], bcB[:], iota, None, eq)

            # hidT accumulated on the PE: query term + 4 histogram
            # chunks (K=128 contracts both partition halves = the
            # position sum; Bm is pre-scaled by 1/8 on the host)
            hidT_ps = pp.tile([H, BS], f32, tag="hid")
            nc.tensor.matmul(hidT_ps[:], cst[0:V, 64:128], qT[:],
                             start=True, stop=False)
            for c in range(4):
                nc.tensor.matmul(hidT_ps[:], cst[:, 0:64],
                                 m8[:, c * BS:(c + 1) * BS],
                                 start=False, stop=(c == 3))

            # hid = relu(hidT + r1_b)   (scalar engine, PSUM -> SBUF)
            hid = pool.tile([H, BS], bf16)
            nc.scalar.activation(hid[:], hidT_ps[:],
                                 mybir.ActivationFunctionType.Relu,
                                 bias=r1b, scale=1.0)

            # logT = r2wT^T @ hid ; + r2_b split across DVE + ACT
            logT_ps = pp.tile([V, BS], f32, tag="log")
            nc.tensor.matmul(logT_ps[:], cst[0:H, 128:192], hid[:],
                             start=True, stop=True)
            logT_sb = pool.tile([V, BS], f32)
            hb = BS // 2
            nc.vector.tensor_scalar(logT_sb[:, 0:hb], logT_ps[:, 0:hb],
                                    r2b, None, add)
            nc.scalar.add(logT_sb[:, hb:BS], logT_ps[:, hb:BS], r2b)
            nc.sync.dma_start(out_d[:, 0:hb], logT_sb[:, 0:hb])
            nc.scalar.dma_start(out_d[:, hb:BS], logT_sb[:, hb:BS])

    nc.compile()
    return nc


def _prep_in_maps(inputs):
    import ml_dtypes
    bf16 = ml_dtypes.bfloat16

    embed = np.asarray(inputs["embed"], dtype=np.float32)[:V]      # (64, 64)
    r1_w = np.asarray(inputs["r1_w"], dtype=np.float32)            # (64, 128)
    r1_b = np.asarray(inputs["r1_b"], dtype=np.float32)            # (64,)
    r2_w = np.asarray(inputs["r2_w"], dtype=np.float32)            # (64, 64)
    r2_b = np.asarray(inputs["r2_b"], dtype=np.float32)            # (64,)
    seqs = np.asarray(inputs["seqs"])                              # (B, 64) int
    query = np.asarray(inputs["query_tok"])                        # (B,) int

    A = embed @ r1_w[:, :H].T                                      # (64v, 64h)
    Bm = (embed @ r1_w[:, H:].T) * np.float32(1.0 / NPOS)          # (64v, 64h)
    cst = np.zeros((2 * H, 256), np.float32)
    cst[0:V, 0:64] = Bm
    cst[V:2 * V, 0:64] = Bm
    cst[0:V, 64:128] = A
    cst[0:H, 128:192] = r2_w.T
    cst16 = cst.astype(bf16).view(np.uint16)
    # biases stay f32: packed as raw bit patterns into bf16 col pairs
    cst16[0:H, 192:194] = r1_b.view(np.uint16).reshape(H, 2)
    cst16[0:V, 194:196] = r2_b.view(np.uint16).reshape(V, 2)
    cst = cst16.view(bf16)

    # single-partition token row, position-major:
    # [pos0..3 | query | pos4..7 offset by +64 | pad]
    tail = seqs[:, TAIL_LO:TAIL_HI].astype(np.float32)             # (B, 8)
    toks = np.zeros((N_CORES, 1, 10 * BS), np.float32)
    for c in range(N_CORES):
        sh = tail[c * BS:(c + 1) * BS]                             # (256, 8)
        toks[c, 0, 0:4 * BS] = sh[:, 0:4].T.reshape(4 * BS)
        toks[c, 0, 4 * BS:5 * BS] = query[c * BS:(c + 1) * BS]
        toks[c, 0, 5 * BS:9 * BS] = sh[:, 4:8].T.reshape(4 * BS) + V
    toks = toks.astype(bf16)

    return [
        {"toks": toks[c], "cst": cst}
        for c in range(N_CORES)
    ]


def kernel(**inputs):
    global _compiled_nc
    from concourse.bass_utils import run_bass_kernel_spmd

    in_maps = _prep_in_maps(inputs)
    if _compiled_nc is None:
        _compiled_nc = _build_program()
    res = run_bass_kernel_spmd(_compiled_nc, in_maps, list(range(N_CORES)))
    out = np.empty((B, V), np.float32)
    for c in range(N_CORES):
        out[c * BS:(c + 1) * BS] = res.results[c]["logT"].T
    return out


if __name__ == "__main__":
    rng = np.random.default_rng(0)
    demo = {
        "embed": rng.standard_normal((V + 2, H)).astype(np.float32),
        "r1_w": rng.standard_normal((H, 2 * H)).astype(np.float32) * 0.05,
        "r1_b": rng.standard_normal(H).astype(np.float32) * 0.02,
        "r2_w": rng.standard_normal((V, H)).astype(np.float32) * 0.05,
        "r2_b": rng.standard_normal(V).astype(np.float32) * 0.02,
        "seqs": rng.integers(0, V, (B, 64)),
        "query_tok": rng.integers(0, V, (B,)),
    }
    out = kernel(**demo)
    tail = demo["embed"][demo["seqs"][:, TAIL_LO:TAIL_HI]]
    mem = tail.sum(1) / NPOS
    h = np.concatenate([demo["embed"][demo["query_tok"]], mem], -1)
    exp = np.maximum(h @ demo["r1_w"].T + demo["r1_b"], 0) @ demo["r2_w"].T + demo["r2_b"]
    err = np.abs(out - exp).max() / np.abs(exp).max()
    print("self-check rel err:", err)


# revision 9
# speedup vs baseline: 1.1105x; 1.0309x over previous
"""Trainium2 Bass kernel for nn_ConsolidationModel.

Mathematical reduction (verified bit-exact against the reference scan):
the scan's control flow is data-independent (count depends only on t).
Consolidation fires at t=15/31/47, but between consecutive firings the
8-slot FIFO receives 4 appends + 12 shift-appends, which evicts every
consolidated row before the next firing — and after the last firing
(t=47) there are 4 appends + 11 shifts, so at t=62 the buffer holds
exactly the embeddings of tokens 55..62 with count=8.  The model output
is therefore:

    mem  = mean_p embed[seqs[:, 55+p]]          (p = 0..7)
    h    = concat([embed[query_tok], mem], -1)  (B, 128)
    out  = relu(h @ r1_w.T + r1_b) @ r2_w.T + r2_b

Device algorithm (per core, batch shard of 256 rows).  Profiling
showed three latency sinks in earlier versions: (1) any Scalar-engine
ACTIVATE triggers an ACT_TABLE_LOAD whose background table DMA parks
SDMA engine 15 until ~11us, so every input DMA's 16th completion
increment straggles ~2.3us behind the other 15; (2) same story for
GpSimd custom ops (iota) via the gpsimd library load; (3) matmuls cost
~165ns + 0.9ns/col, so many small matmuls serialize badly.  v3:

  wide (128, 1280) i16 <- two stride-0 broadcast DMAs (sync queue):
                          top 64 partitions = tail positions 0..3
                          (position-major) + query; bottom 64 =
                          positions 4..7 offset +64, query slot = -1
                          sentinel.  128 partitions engage all 16
                          SDMA engines (engines are partition-pinned).
  cst  (128, 256) bf16 <- one DMA (scalar queue): [Bm;Bm], [A;0],
                          r2w.T, biases + 0..127 iota packed as raw
                          f32 bit patterns in bf16 column pairs.
  m8   = (wide == iota) ONE is_equal tensor_scalar, i16 SBUF 4x mode
  hist = chunk pre-adds (3 bf16 2x tensor_tensor ops)
  hidT = [A;0]^T @ m8_query  +  [Bm;Bm]^T @ hist   (2 bf16 matmuls,
         K=128 contracts both partition halves = the position sum)
  hid  = relu(hidT + r1b)  fused 2-op DVE tensor_scalar (add, max)
  logT = r2wT^T @ hid ; + r2b — split into batch halves pipelined
         through PE/DVE so the two 32KB output DMAs (sync + scalar
         queues) issue as early as possible.

No Scalar-engine ACTIVATE, no GpSimd ops anywhere.

Sharding: pure data parallel over batch across 8 cores; parameters
replicated.  Host transposes each (64, 256) shard back to (256, 64).
"""

import numpy as np

N_CORES = 8
B = 2048           # full batch
BS = B // N_CORES  # 256 per-core batch shard
H = 64             # hidden dim
V = 64             # vocab
TAIL_LO, TAIL_HI = 55, 63  # token positions that survive in the buffer
NPOS = TAIL_HI - TAIL_LO   # 8

_compiled_nc = None


def _build_program():
    import concourse.bacc as bacc
    import concourse.mybir as mybir
    from concourse import tile

    f32 = mybir.dt.float32
    bf16 = mybir.dt.bfloat16
    i16 = mybir.dt.int16
    eq = mybir.AluOpType.is_equal
    add = mybir.AluOpType.add
    mx = mybir.AluOpType.max

    nc = bacc.Bacc("TRN2", target_bir_lowering=False, debug=False,
                   num_devices=N_CORES)

    wide_d = nc.declare_dram_parameter("wide", [2, 5 * BS], i16, isOutput=False)
    cst_d = nc.declare_dram_parameter("cst", [2 * H, 256], bf16, isOutput=False)
    out_d = nc.declare_dram_parameter("logT", [V, BS], f32, isOutput=True)

    with tile.TileContext(nc) as tc:
        with (
            tc.tile_pool(name="sb", bufs=1) as pool,
            tc.tile_pool(name="ps", bufs=1, space="PSUM") as pp,
        ):
            # token broadcast: two 64-partition stride-0 DMAs on the
            # sync queue; constants on the scalar queue in parallel
            wide = pool.tile([2 * V, 5 * BS], i16)
            nc.sync.dma_start(wide[0:V, :],
                              wide_d[0:1, :].to_broadcast((V, 5 * BS)))
            nc.sync.dma_start(wide[V:2 * V, :],
                              wide_d[1:2, :].to_broadcast((V, 5 * BS)))
            cst = pool.tile([2 * H, 256], bf16)
            nc.scalar.dma_start(cst[:], cst_d[:])
            # f32 values packed as bit patterns into bf16 column pairs
            r1b = cst[0:H, 192:194].bitcast(f32)
            r2b = cst[0:V, 194:196].bitcast(f32)
            iota = cst[0:2 * V, 196:198].bitcast(f32)

            # one-hot masks: ONE is_equal over all 8 positions + query
            # (i16 SBUF source -> DVE 4x mode)
            m8 = pool.tile([2 * V, 5 * BS], bf16)
            nc.vector.tensor_scalar(m8[:], wide[:], iota, None, eq)

            # histogram: bf16 2x pre-adds over the 4 position chunks
            s01 = pool.tile([2 * V, BS], bf16)
            nc.vector.tensor_add(s01[:], m8[:, 0:BS], m8[:, BS:2 * BS])
            s23 = pool.tile([2 * V, BS], bf16)
            nc.vector.tensor_add(s23[:], m8[:, 2 * BS:3 * BS],
                                 m8[:, 3 * BS:4 * BS])
            hist = pool.tile([2 * V, BS], bf16)
            nc.vector.tensor_add(hist[:], s01[:], s23[:])

            # hidT = [A;0]^T @ q1h + [Bm;Bm]^T @ hist
            hidT_ps = pp.tile([H, BS], f32, tag="hid")
            nc.tensor.matmul(hidT_ps[:], cst[:, 64:128],
                             m8[:, 4 * BS:5 * BS], start=True, stop=False)
            nc.tensor.matmul(hidT_ps[:], cst[:, 0:64], hist[:],
                             start=False, stop=True)

            # tail pipelined in batch halves: relu -> logT matmul ->
            # +r2b -> DMA out, so the first DMA issues early
            hid = pool.tile([H, BS], bf16)
            logT_ps = pp.tile([V, BS], f32, tag="log")
            logT_sb = pool.tile([V, BS], f32)
            hb = BS // 2
            for lo, hi, dma in ((0, hb, nc.sync), (hb, BS, nc.scalar)):
                nc.vector.tensor_scalar(hid[:, lo:hi], hidT_ps[:, lo:hi],
                                        r1b, 0.0, add, mx)
                nc.tensor.matmul(logT_ps[:, lo:hi], cst[0:H, 128:192],
                                 hid[:, lo:hi], start=True, stop=True)
                nc.vector.tensor_scalar(logT_sb[:, lo:hi], logT_ps[:, lo:hi],
                                        r2b, None, add)
                dma.dma_start(out_d[:, lo:hi], logT_sb[:, lo:hi])

    nc.compile()
    return nc


def _prep_in_maps(inputs):
    import ml_dtypes
    bft = ml_dtypes.bfloat16

    embed = np.asarray(inputs["embed"], dtype=np.float32)[:V]      # (64, 64)
    r1_w = np.asarray(inputs["r1_w"], dtype=np.float32)            # (64, 128)
    r1_b = np.asarray(inputs["r1_b"], dtype=np.float32)            # (64,)
    r2_w = np.asarray(inputs["r2_w"], dtype=np.float32)            # (64, 64)
    r2_b = np.asarray(inputs["r2_b"], dtype=np.float32)            # (64,)
    seqs = np.asarray(inputs["seqs"])                              # (B, 64) int
    query = np.asarray(inputs["query_tok"])                        # (B,) int

    A = embed @ r1_w[:, :H].T                                      # (64v, 64h)
    Bm = (embed @ r1_w[:, H:].T) * np.float32(1.0 / NPOS)          # (64v, 64h)
    cst = np.zeros((2 * H, 256), np.float32)
    cst[0:V, 0:64] = Bm
    cst[V:2 * V, 0:64] = Bm
    cst[0:V, 64:128] = A
    cst[0:H, 128:192] = r2_w.T
    cst16 = cst.astype(bft).view(np.uint16)
    # f32 values packed as raw bit patterns into bf16 column pairs
    cst16[0:H, 192:194] = r1_b.view(np.uint16).reshape(H, 2)
    cst16[0:V, 194:196] = r2_b.view(np.uint16).reshape(V, 2)
    iota = np.arange(2 * V, dtype=np.float32)
    cst16[:, 196:198] = iota.view(np.uint16).reshape(2 * V, 2)
    cst = cst16.view(bft)

    # broadcast rows, position-major: row0 = pos0..3 + query,
    # row1 = pos4..7 offset +64, query slot = -1 sentinel
    tail = seqs[:, TAIL_LO:TAIL_HI].astype(np.int16)               # (B, 8)
    wide = np.empty((N_CORES, 2, 5 * BS), np.int16)
    for c in range(N_CORES):
        sh = tail[c * BS:(c + 1) * BS]                             # (256, 8)
        wide[c, 0, 0:4 * BS] = sh[:, 0:4].T.reshape(4 * BS)
        wide[c, 0, 4 * BS:5 * BS] = query[c * BS:(c + 1) * BS]
        wide[c, 1, 0:4 * BS] = sh[:, 4:8].T.reshape(4 * BS) + V
        wide[c, 1, 4 * BS:5 * BS] = -1
    return [
        {"wide": wide[c], "cst": cst}
        for c in range(N_CORES)
    ]


def kernel(**inputs):
    global _compiled_nc
    from concourse.bass_utils import run_bass_kernel_spmd

    in_maps = _prep_in_maps(inputs)
    if _compiled_nc is None:
        _compiled_nc = _build_program()
    res = run_bass_kernel_spmd(_compiled_nc, in_maps, list(range(N_CORES)))
    out = np.empty((B, V), np.float32)
    for c in range(N_CORES):
        out[c * BS:(c + 1) * BS] = res.results[c]["logT"].T
    return out


if __name__ == "__main__":
    rng = np.random.default_rng(0)
    demo = {
        "embed": rng.standard_normal((V + 2, H)).astype(np.float32),
        "r1_w": rng.standard_normal((H, 2 * H)).astype(np.float32) * 0.05,
        "r1_b": rng.standard_normal(H).astype(np.float32) * 0.02,
        "r2_w": rng.standard_normal((V, H)).astype(np.float32) * 0.05,
        "r2_b": rng.standard_normal(V).astype(np.float32) * 0.02,
        "seqs": rng.integers(0, V, (B, 64)),
        "query_tok": rng.integers(0, V, (B,)),
    }
    out = kernel(**demo)
    tail = demo["embed"][demo["seqs"][:, TAIL_LO:TAIL_HI]]
    mem = tail.sum(1) / NPOS
    h = np.concatenate([demo["embed"][demo["query_tok"]], mem], -1)
    exp = np.maximum(h @ demo["r1_w"].T + demo["r1_b"], 0) @ demo["r2_w"].T + demo["r2_b"]
    err = np.abs(out - exp).max() / np.abs(exp).max()
    print("self-check rel err:", err)


# revision 10
# speedup vs baseline: 1.1642x; 1.0483x over previous
"""Trainium2 Bass kernel for nn_ConsolidationModel.

Mathematical reduction (verified bit-exact against the reference scan):
the scan's control flow is data-independent (count depends only on t).
Consolidation fires at t=15/31/47, but between consecutive firings the
8-slot FIFO receives 4 appends + 12 shift-appends, which evicts every
consolidated row before the next firing — and after the last firing
(t=47) there are 4 appends + 11 shifts, so at t=62 the buffer holds
exactly the embeddings of tokens 55..62 with count=8.  The model output
is therefore:

    mem  = mean_p embed[seqs[:, 55+p]]          (p = 0..7)
    h    = concat([embed[query_tok], mem], -1)  (B, 128)
    out  = relu(h @ r1_w.T + r1_b) @ r2_w.T + r2_b

Device algorithm (per core, batch shard of 256 rows).  Profiling
learnings baked in: (1) SDMA is descriptor-bound at ~180ns/descriptor/
engine (stride-0 broadcast sources are ~2x worse), so ALL input ships
as ONE host-replicated tensor = 128 large descriptors on one queue;
(2) Scalar-engine ACTIVATE and GpSimd custom ops each trigger
background table/library DMAs that park an SDMA engine and straggle
every DMA completion by ~2.3us, so neither engine executes any compute
op; (3) matmuls cost ~165ns + 0.9ns/col, so the histogram is pre-added
on the DVE (bf16 2x) down to 2 accumulated matmuls.

  wide (128, 1480) i16 <- ONE DMA: cols 0:1280 = tokens broadcast
       (rows replicated on host: top half = tail positions 0..3
       position-major + query; bottom half = positions 4..7 offset
       +64, query slot = -1 sentinel), then bit-packed constants:
       iota f32, r1b/r2b f32, [Bm;Bm]/[A;0]/r2w.T bf16 — all read on
       device via bitcast APs.
  m8   = (wide == iota) ONE is_equal tensor_scalar (i16 SBUF, 4x DVE)
  hist = 3 bf16 2x tensor_tensor pre-adds over the 4 position chunks
  hidT = [A;0]^T @ m8_query + [Bm;Bm]^T @ hist  (2 bf16 matmuls,
         K=128 contracts both partition halves = the position sum)
  hid  = relu(hidT + r1b)  fused 2-op DVE tensor_scalar (add, max)
  logT = r2wT^T @ hid ; + r2b — pipelined in batch halves so the two
         32KB output DMAs (sync + scalar queues) issue early.

Sharding: pure data parallel over batch across 8 cores; parameters
replicated.  Host transposes each (64, 256) shard back to (256, 64).
"""

import numpy as np

N_CORES = 8
B = 2048           # full batch
BS = B // N_CORES  # 256 per-core batch shard
H = 64             # hidden dim
V = 64             # vocab
TAIL_LO, TAIL_HI = 55, 63  # token positions that survive in the buffer
NPOS = TAIL_HI - TAIL_LO   # 8

# wide tensor column map (i16 columns)
C_TOK = 0          # 0:1280   tokens (4 position-pair chunks + query)
C_IOTA = 5 * BS          # 1280:1282  iota 0..127 as packed f32
C_R1B = C_IOTA + 2       # 1282:1284  r1_b as packed f32 (rows 0:64)
C_R2B = C_R1B + 2        # 1284:1286  r2_b as packed f32 (rows 0:64)
C_BM = C_R2B + 2         # 1286:1350  [Bm;Bm] bf16 bits
C_A = C_BM + H           # 1350:1414  [A;0] bf16 bits
C_R2W = C_A + H          # 1414:1478  [r2w.T;0] bf16 bits
C_END = C_R2W + H + 2    # 1480 (pad to keep 4B-aligned total)

_compiled_nc = None


def _build_program():
    import concourse.bacc as bacc
    import concourse.mybir as mybir
    from concourse import tile

    f32 = mybir.dt.float32
    bf16 = mybir.dt.bfloat16
    i16 = mybir.dt.int16
    eq = mybir.AluOpType.is_equal
    add = mybir.AluOpType.add
    mx = mybir.AluOpType.max

    nc = bacc.Bacc("TRN2", target_bir_lowering=False, debug=False,
                   num_devices=N_CORES)

    wide_d = nc.declare_dram_parameter("wide", [2 * H, C_END], i16,
                                       isOutput=False)
    out_d = nc.declare_dram_parameter("logT", [V, BS], f32, isOutput=True)

    with tile.TileContext(nc) as tc:
        with (
            tc.tile_pool(name="sb", bufs=1) as pool,
            tc.tile_pool(name="ps", bufs=1, space="PSUM") as pp,
        ):
            # the whole input: one DMA, one semaphore, 128 descriptors
            wide = pool.tile([2 * H, C_END], i16)
            nc.sync.dma_start(wide[:], wide_d[:])

            iota = wide[:, C_IOTA:C_IOTA + 2].bitcast(f32)
            r1b = wide[0:H, C_R1B:C_R1B + 2].bitcast(f32)
            r2b = wide[0:V, C_R2B:C_R2B + 2].bitcast(f32)
            w_bm = wide[:, C_BM:C_BM + H].bitcast(bf16)
            w_a = wide[:, C_A:C_A + H].bitcast(bf16)
            w_r2 = wide[0:H, C_R2W:C_R2W + H].bitcast(bf16)

            # one-hot masks: ONE is_equal over all 8 positions + query
            m8 = pool.tile([2 * V, 5 * BS], bf16)
            nc.vector.tensor_scalar(m8[:], wide[:, 0:5 * BS], iota, None, eq)

            # histogram: bf16 2x pre-adds over the 4 position chunks
            s01 = pool.tile([2 * V, BS], bf16)
            nc.vector.tensor_add(s01[:], m8[:, 0:BS], m8[:, BS:2 * BS])
            s23 = pool.tile([2 * V, BS], bf16)
            nc.vector.tensor_add(s23[:], m8[:, 2 * BS:3 * BS],
                                 m8[:, 3 * BS:4 * BS])
            hist = pool.tile([2 * V, BS], bf16)
            nc.vector.tensor_add(hist[:], s01[:], s23[:])

            # hidT = [A;0]^T @ q1h + [Bm;Bm]^T @ hist
            hidT_ps = pp.tile([H, BS], f32, tag="hid")
            nc.tensor.matmul(hidT_ps[:], w_a, m8[:, 4 * BS:5 * BS],
                             start=True, stop=False)
            nc.tensor.matmul(hidT_ps[:], w_bm, hist[:],
                             start=False, stop=True)

            # tail pipelined in batch halves: relu -> logT matmul ->
            # +r2b -> DMA out, so the first DMA issues early
            hid = pool.tile([H, BS], bf16)
            logT_ps = pp.tile([V, BS], f32, tag="log")
            logT_sb = pool.tile([V, BS], f32)
            hb = BS // 2
            for lo, hi, dma in ((0, hb, nc.sync), (hb, BS, nc.scalar)):
                nc.vector.tensor_scalar(hid[:, lo:hi], hidT_ps[:, lo:hi],
                                        r1b, 0.0, add, mx)
                nc.tensor.matmul(logT_ps[:, lo:hi], w_r2, hid[:, lo:hi],
                                 start=True, stop=True)
                nc.vector.tensor_scalar(logT_sb[:, lo:hi], logT_ps[:, lo:hi],
                                        r2b, None, add)
                dma.dma_start(out_d[:, lo:hi], logT_sb[:, lo:hi])

    nc.compile()
    return nc


def _prep_in_maps(inputs):
    import ml_dtypes
    bft = ml_dtypes.bfloat16

    embed = np.asarray(inputs["embed"], dtype=np.float32)[:V]      # (64, 64)
    r1_w = np.asarray(inputs["r1_w"], dtype=np.float32)            # (64, 128)
    r1_b = np.asarray(inputs["r1_b"], dtype=np.float32)            # (64,)
    r2_w = np.asarray(inputs["r2_w"], dtype=np.float32)            # (64, 64)
    r2_b = np.asarray(inputs["r2_b"], dtype=np.float32)            # (64,)
    seqs = np.asarray(inputs["seqs"])                              # (B, 64) int
    query = np.asarray(inputs["query_tok"])                        # (B,) int

    A = embed @ r1_w[:, :H].T                                      # (64v, 64h)
    Bm = (embed @ r1_w[:, H:].T) * np.float32(1.0 / NPOS)          # (64v, 64h)

    # constant columns (identical for every core), as i16 bit patterns
    cc = np.zeros((2 * H, C_END - C_IOTA), np.uint16)
    iota = np.arange(2 * V, dtype=np.float32)
    cc[:, 0:2] = iota.view(np.uint16).reshape(2 * V, 2)
    cc[0:H, 2:4] = r1_b.view(np.uint16).reshape(H, 2)
    cc[0:V, 4:6] = r2_b.view(np.uint16).reshape(V, 2)
    cc[0:V, 6:6 + H] = Bm.astype(bft).view(np.uint16)
    cc[V:2 * V, 6:6 + H] = cc[0:V, 6:6 + H]
    cc[0:V, 6 + H:6 + 2 * H] = A.astype(bft).view(np.uint16)
    cc[0:H, 6 + 2 * H:6 + 3 * H] = r2_w.T.astype(bft).view(np.uint16)

    # token region: rows replicated on host (no stride-0 descriptors)
    tail = seqs[:, TAIL_LO:TAIL_HI].astype(np.int16)               # (B, 8)
    wide = np.empty((N_CORES, 2 * H, C_END), np.int16)
    wide[:, :, C_IOTA:] = cc.view(np.int16)
    for c in range(N_CORES):
        sh = tail[c * BS:(c + 1) * BS]                             # (256, 8)
        row0 = np.empty(5 * BS, np.int16)
        row0[0:4 * BS] = sh[:, 0:4].T.reshape(4 * BS)
        row0[4 * BS:5 * BS] = query[c * BS:(c + 1) * BS]
        row1 = np.empty(5 * BS, np.int16)
        row1[0:4 * BS] = sh[:, 4:8].T.reshape(4 * BS) + V
        row1[4 * BS:5 * BS] = -1
        wide[c, 0:V, 0:5 * BS] = row0
        wide[c, V:2 * V, 0:5 * BS] = row1
    return [{"wide": wide[c]} for c in range(N_CORES)]


def kernel(**inputs):
    global _compiled_nc
    from concourse.bass_utils import run_bass_kernel_spmd

    in_maps = _prep_in_maps(inputs)
    if _compiled_nc is None:
        _compiled_nc = _build_program()
    res = run_bass_kernel_spmd(_compiled_nc, in_maps, list(range(N_CORES)))
    out = np.empty((B, V), np.float32)
    for c in range(N_CORES):
        out[c * BS:(c + 1) * BS] = res.results[c]["logT"].T
    return out


if __name__ == "__main__":
    rng = np.random.default_rng(0)
    demo = {
        "embed": rng.standard_normal((V + 2, H)).astype(np.float32),
        "r1_w": rng.standard_normal((H, 2 * H)).astype(np.float32) * 0.05,
        "r1_b": rng.standard_normal(H).astype(np.float32) * 0.02,
        "r2_w": rng.standard_normal((V, H)).astype(np.float32) * 0.05,
        "r2_b": rng.standard_normal(V).astype(np.float32) * 0.02,
        "seqs": rng.integers(0, V, (B, 64)),
        "query_tok": rng.integers(0, V, (B,)),
    }
    out = kernel(**demo)
    tail = demo["embed"][demo["seqs"][:, TAIL_LO:TAIL_HI]]
    mem = tail.sum(1) / NPOS
    h = np.concatenate([demo["embed"][demo["query_tok"]], mem], -1)
    exp = np.maximum(h @ demo["r1_w"].T + demo["r1_b"], 0) @ demo["r2_w"].T + demo["r2_b"]
    err = np.abs(out - exp).max() / np.abs(exp).max()
    print("self-check rel err:", err)


# revision 11
# speedup vs baseline: 1.1940x; 1.0256x over previous
"""Trainium2 Bass kernel for nn_ConsolidationModel.

Mathematical reduction (verified bit-exact against the reference scan):
the scan's control flow is data-independent (count depends only on t).
Consolidation fires at t=15/31/47, but between consecutive firings the
8-slot FIFO receives 4 appends + 12 shift-appends, which evicts every
consolidated row before the next firing — and after the last firing
(t=47) there are 4 appends + 11 shifts, so at t=62 the buffer holds
exactly the embeddings of tokens 55..62 with count=8.  The model output
is therefore:

    mem  = mean_p embed[seqs[:, 55+p]]          (p = 0..7)
    h    = concat([embed[query_tok], mem], -1)  (B, 128)
    out  = relu(h @ r1_w.T + r1_b) @ r2_w.T + r2_b

Device algorithm (per core, batch shard of 256 rows).  Profiling
learnings baked in: (1) SDMA is descriptor-bound at ~180ns/descriptor/
engine (stride-0 broadcast sources are ~2x worse), so ALL input ships
as ONE host-replicated tensor = 128 large descriptors on one queue;
(2) Scalar-engine ACTIVATE and GpSimd custom ops each trigger
background table/library DMAs that park an SDMA engine and straggle
every DMA completion by ~2.3us, so neither engine executes any compute
op; (3) matmuls cost ~165ns + 0.9ns/col, so the histogram is pre-added
on the DVE (bf16 2x) down to 2 accumulated matmuls.

  wide (128, 1480) i16 <- ONE DMA: cols 0:1280 = tokens broadcast
       (rows replicated on host: top half = tail positions 0..3
       position-major + query; bottom half = positions 4..7 offset
       +64, query slot = -1 sentinel), then bit-packed constants:
       iota f32, r1b/r2b f32, [Bm;Bm]/[A;0]/r2w.T bf16 — all read on
       device via bitcast APs.
  m8   = (wide == iota) ONE is_equal tensor_scalar (i16 SBUF, 4x DVE)
  hist = 3 bf16 2x tensor_tensor pre-adds over the 4 position chunks
  hidT = [A;0]^T @ m8_query + [Bm;Bm]^T @ hist  (2 bf16 matmuls,
         K=128 contracts both partition halves = the position sum)
  hid  = relu(hidT + r1b)  fused 2-op DVE tensor_scalar (add, max)
  logT = r2wT^T @ hid ; + r2b — pipelined in batch halves so the two
         32KB output DMAs (sync + scalar queues) issue early.

Sharding: pure data parallel over batch across 8 cores; parameters
replicated.  Host transposes each (64, 256) shard back to (256, 64).
"""

import numpy as np

N_CORES = 8
B = 2048           # full batch
BS = B // N_CORES  # 256 per-core batch shard
H = 64             # hidden dim
V = 64             # vocab
TAIL_LO, TAIL_HI = 55, 63  # token positions that survive in the buffer
NPOS = TAIL_HI - TAIL_LO   # 8

# wide tensor column map (i16 columns)
C_TOK = 0          # 0:1280   tokens (4 position-pair chunks + query)
C_IOTA = 5 * BS          # 1280:1282  iota 0..127 as packed f32
C_R1B = C_IOTA + 2       # 1282:1284  r1_b as packed f32 (rows 0:64)
C_R2B = C_R1B + 2        # 1284:1286  r2_b as packed f32 (rows 0:64)
C_BM = C_R2B + 2         # 1286:1350  [Bm;Bm] bf16 bits
C_A = C_BM + H           # 1350:1414  [A;0] bf16 bits
C_R2W = C_A + H          # 1414:1478  [r2w.T;0] bf16 bits
C_END = C_R2W + H + 2    # 1480 (pad to keep 4B-aligned total)

_compiled_nc = None


def _build_program():
    import concourse.bacc as bacc
    import concourse.mybir as mybir
    from concourse import tile

    f32 = mybir.dt.float32
    bf16 = mybir.dt.bfloat16
    i16 = mybir.dt.int16
    eq = mybir.AluOpType.is_equal
    add = mybir.AluOpType.add
    mx = mybir.AluOpType.max

    nc = bacc.Bacc("TRN2", target_bir_lowering=False, debug=False,
                   num_devices=N_CORES)

    wide_d = nc.declare_dram_parameter("wide", [2 * H, C_END], i16,
                                       isOutput=False)
    out_d = nc.declare_dram_parameter("logT", [V, BS], f32, isOutput=True)

    with tile.TileContext(nc) as tc:
        with (
            tc.tile_pool(name="sb", bufs=1) as pool,
            tc.tile_pool(name="ps", bufs=1, space="PSUM") as pp,
        ):
            # the whole input: one DMA, one semaphore, 128 descriptors
            wide = pool.tile([2 * H, C_END], i16)
            nc.sync.dma_start(wide[:], wide_d[:])

            iota = wide[:, C_IOTA:C_IOTA + 2].bitcast(f32)
            r1b = wide[0:H, C_R1B:C_R1B + 2].bitcast(f32)
            r2b = wide[0:V, C_R2B:C_R2B + 2].bitcast(f32)
            w_bm = wide[:, C_BM:C_BM + H].bitcast(bf16)
            w_a = wide[:, C_A:C_A + H].bitcast(bf16)
            w_r2 = wide[0:H, C_R2W:C_R2W + H].bitcast(bf16)

            # one-hot masks: ONE is_equal over all 8 positions + query
            m8 = pool.tile([2 * V, 5 * BS], bf16)
            nc.vector.tensor_scalar(m8[:], wide[:, 0:5 * BS], iota, None, eq)

            # histogram: 2-level bf16 2x fold over the 4 position chunks
            s2 = pool.tile([2 * V, 2 * BS], bf16)
            nc.vector.tensor_add(s2[:], m8[:, 0:2 * BS], m8[:, 2 * BS:4 * BS])
            hist = pool.tile([2 * V, BS], bf16)
            nc.vector.tensor_add(hist[:], s2[:, 0:BS], s2[:, BS:2 * BS])

            # hidT = [A;0]^T @ q1h + [Bm;Bm]^T @ hist
            hidT_ps = pp.tile([H, BS], f32, tag="hid")
            nc.tensor.matmul(hidT_ps[:], w_a, m8[:, 4 * BS:5 * BS],
                             start=True, stop=False)
            nc.tensor.matmul(hidT_ps[:], w_bm, hist[:],
                             start=False, stop=True)

            # tail pipelined in batch halves: relu -> logT matmul ->
            # +r2b -> DMA out, so the first DMA issues early
            hid = pool.tile([H, BS], bf16)
            logT_ps = pp.tile([V, BS], f32, tag="log")
            logT_sb = pool.tile([V, BS], f32)
            hb = BS // 2
            for lo, hi, dma in ((0, hb, nc.sync), (hb, BS, nc.scalar)):
                nc.vector.tensor_scalar(hid[:, lo:hi], hidT_ps[:, lo:hi],
                                        r1b, 0.0, add, mx)
                nc.tensor.matmul(logT_ps[:, lo:hi], w_r2, hid[:, lo:hi],
                                 start=True, stop=True)
                nc.vector.tensor_scalar(logT_sb[:, lo:hi], logT_ps[:, lo:hi],
                                        r2b, None, add)
                dma.dma_start(out_d[:, lo:hi], logT_sb[:, lo:hi])

    nc.compile()
    return nc


def _prep_in_maps(inputs):
    import ml_dtypes
    bft = ml_dtypes.bfloat16

    embed = np.asarray(inputs["embed"], dtype=np.float32)[:V]      # (64, 64)
    r1_w = np.asarray(inputs["r1_w"], dtype=np.float32)            # (64, 128)
    r1_b = np.asarray(inputs["r1_b"], dtype=np.float32)            # (64,)
    r2_w = np.asarray(inputs["r2_w"], dtype=np.float32)            # (64, 64)
    r2_b = np.asarray(inputs["r2_b"], dtype=np.float32)            # (64,)
    seqs = np.asarray(inputs["seqs"])                              # (B, 64) int
    query = np.asarray(inputs["query_tok"])                        # (B,) int

    A = embed @ r1_w[:, :H].T                                      # (64v, 64h)
    Bm = (embed @ r1_w[:, H:].T) * np.float32(1.0 / NPOS)          # (64v, 64h)

    # constant columns (identical for every core), as i16 bit patterns
    cc = np.zeros((2 * H, C_END - C_IOTA), np.uint16)
    iota = np.arange(2 * V, dtype=np.float32)
    cc[:, 0:2] = iota.view(np.uint16).reshape(2 * V, 2)
    cc[0:H, 2:4] = r1_b.view(np.uint16).reshape(H, 2)
    cc[0:V, 4:6] = r2_b.view(np.uint16).reshape(V, 2)
    cc[0:V, 6:6 + H] = Bm.astype(bft).view(np.uint16)
    cc[V:2 * V, 6:6 + H] = cc[0:V, 6:6 + H]
    cc[0:V, 6 + H:6 + 2 * H] = A.astype(bft).view(np.uint16)
    cc[0:H, 6 + 2 * H:6 + 3 * H] = r2_w.T.astype(bft).view(np.uint16)

    # token region: rows replicated on host (no stride-0 descriptors)
    tail = seqs[:, TAIL_LO:TAIL_HI].astype(np.int16)               # (B, 8)
    wide = np.empty((N_CORES, 2 * H, C_END), np.int16)
    wide[:, :, C_IOTA:] = cc.view(np.int16)
    for c in range(N_CORES):
        sh = tail[c * BS:(c + 1) * BS]                             # (256, 8)
        row0 = np.empty(5 * BS, np.int16)
        row0[0:4 * BS] = sh[:, 0:4].T.reshape(4 * BS)
        row0[4 * BS:5 * BS] = query[c * BS:(c + 1) * BS]
        row1 = np.empty(5 * BS, np.int16)
        row1[0:4 * BS] = sh[:, 4:8].T.reshape(4 * BS) + V
        row1[4 * BS:5 * BS] = -1
        wide[c, 0:V, 0:5 * BS] = row0
        wide[c, V:2 * V, 0:5 * BS] = row1
    return [{"wide": wide[c]} for c in range(N_CORES)]


def kernel(**inputs):
    global _compiled_nc
    from concourse.bass_utils import run_bass_kernel_spmd

    in_maps = _prep_in_maps(inputs)
    if _compiled_nc is None:
        _compiled_nc = _build_program()
    res = run_bass_kernel_spmd(_compiled_nc, in_maps, list(range(N_CORES)))
    out = np.empty((B, V), np.float32)
    for c in range(N_CORES):
        out[c * BS:(c + 1) * BS] = res.results[c]["logT"].T
    return out


if __name__ == "__main__":
    rng = np.random.default_rng(0)
    demo = {
        "embed": rng.standard_normal((V + 2, H)).astype(np.float32),
        "r1_w": rng.standard_normal((H, 2 * H)).astype(np.float32) * 0.05,
        "r1_b": rng.standard_normal(H).astype(np.float32) * 0.02,
        "r2_w": rng.standard_normal((V, H)).astype(np.float32) * 0.05,
        "r2_b": rng.standard_normal(V).astype(np.float32) * 0.02,
        "seqs": rng.integers(0, V, (B, 64)),
        "query_tok": rng.integers(0, V, (B,)),
    }
    out = kernel(**demo)
    tail = demo["embed"][demo["seqs"][:, TAIL_LO:TAIL_HI]]
    mem = tail.sum(1) / NPOS
    h = np.concatenate([demo["embed"][demo["query_tok"]], mem], -1)
    exp = np.maximum(h @ demo["r1_w"].T + demo["r1_b"], 0) @ demo["r2_w"].T + demo["r2_b"]
    err = np.abs(out - exp).max() / np.abs(exp).max()
    print("self-check rel err:", err)
